# revision 5
# baseline (speedup 1.0000x reference)
"""Trainium2 Bass kernel for nn_AttentionSampleUpdater (gnn message passing).

Per node i: pool = candidates from neighbors' sample lists + own samples
(72 candidates); score each by x[i].x[c]; output the top-8 distinct
candidate values by similarity (descending).

Strategy: data-parallel over the node dimension across 8 NeuronCores.
Each core holds the full x table + full current_samples table in DRAM
(for gathers) and processes a 12500-node shard (padded to 98*128).
Per 128-node tile:
  - indirect-DMA gather of neighbor sample rows -> 72 candidate indices
  - indirect-DMA gather of candidate embeddings [128, 72, 32]
  - DVE: elementwise mult + grouped reduce -> sims [128, 72]
  - DVE max8/match_replace rounds -> top-16 sims; adjacent-equal dedupe;
    final max8 -> top-8 distinct sims
  - value extraction: eq-match sims against top-8 + min-reduce over
    (value - 2^23), exact for integer values < 2^24
"""
import numpy as np

N_CORES = 8
N_TOTAL = 100000
D_EMB = 32
K = 8
D_NB = 8
P = D_NB * K + K  # 72
NODES_PER_CORE_RAW = N_TOTAL // N_CORES  # 12500
TILE_ROWS = 128
N_TILES = (NODES_PER_CORE_RAW + TILE_ROWS - 1) // TILE_ROWS  # 98
NODES_PER_CORE = N_TILES * TILE_ROWS  # 12544

_CACHE = {}


def _build(n_total, nodes_per_core, n_cores):
    import concourse.bacc as bacc
    import concourse.mybir as mybir
    from concourse import tile
    from concourse.bass import IndirectOffsetOnAxis

    f32 = mybir.dt.float32
    i32 = mybir.dt.int32
    Alu = mybir.AluOpType
    BIG = float(2 ** 23)
    NEG = -1.0e30

    nc = bacc.Bacc(
        "TRN2", target_bir_lowering=False, debug=False, num_devices=n_cores
    )
    x = nc.dram_tensor("x", (n_total, D_EMB), f32, kind="ExternalInput").ap()
    csf = nc.dram_tensor("cs_full", (n_total, K), i32, kind="ExternalInput").ap()
    nb = nc.dram_tensor("nb", (nodes_per_core, D_NB), i32, kind="ExternalInput").ap()
    cs_loc = nc.dram_tensor("cs_loc", (nodes_per_core, K), i32, kind="ExternalInput").ap()
    x_loc = nc.dram_tensor("x_loc", (nodes_per_core, D_EMB), f32, kind="ExternalInput").ap()
    out = nc.dram_tensor("out", (nodes_per_core, K), i32, kind="ExternalOutput").ap()

    n_tiles = nodes_per_core // TILE_ROWS

    with tile.TileContext(nc) as tc:
        with tc.tile_pool(name="p", bufs=3) as pool, tc.tile_pool(
            name="pbig", bufs=2
        ) as pbig:
            for t in range(n_tiles):
                r = slice(t * TILE_ROWS, (t + 1) * TILE_ROWS)
                nb_t = pool.tile([128, D_NB], i32, tag="nb")
                nc.sync.dma_start(out=nb_t[:], in_=nb[r, :])
                cand = pool.tile([128, P], i32, tag="cand")
                nc.sync.dma_start(out=cand[:, D_NB * K :], in_=cs_loc[r, :])
                # candidate indices: samples of each neighbor (one indirect
                # DMA per neighbor slot; offsets are one-per-partition)
                for j in range(D_NB):
                    nc.gpsimd.indirect_dma_start(
                        out=cand[:, j * K : (j + 1) * K],
                        out_offset=None,
                        in_=csf[:, :],
                        in_offset=IndirectOffsetOnAxis(ap=nb_t[:, j : j + 1], axis=0),
                    )
                xi = pool.tile([128, D_EMB], f32, tag="xi")
                nc.sync.dma_start(out=xi[:], in_=x_loc[r, :])
                # candidate embeddings
                emb = pbig.tile([128, P, D_EMB], f32, tag="emb")
                for c in range(P):
                    nc.gpsimd.indirect_dma_start(
                        out=emb[:, c, :],
                        out_offset=None,
                        in_=x[:, :],
                        in_offset=IndirectOffsetOnAxis(ap=cand[:, c : c + 1], axis=0),
                    )
                # sims = sum over d of emb * x_i
                prod = pbig.tile([128, P, D_EMB], f32, tag="prod")
                nc.vector.tensor_tensor(
                    out=prod[:, :, :],
                    in0=emb[:, :, :],
                    in1=xi[:, None, :].to_broadcast([128, P, D_EMB]),
                    op=Alu.mult,
                )
                sims = pool.tile([128, P], f32, tag="sims")
                nc.vector.tensor_reduce(
                    out=sims[:],
                    in_=prod[:, :, :],
                    axis=mybir.AxisListType.X,
                    op=Alu.add,
                )
                # top-24 sims via three max8 rounds (window covers nodes with
                # up to 16 duplicate-excess candidates, e.g. repeated
                # neighbors duplicating a whole 8-sample block)
                W = 24
                m = pool.tile([128, W], f32, tag="m")
                nc.vector.max(out=m[:, :8], in_=sims[:])
                sims2 = pool.tile([128, P], f32, tag="sims2")
                nc.vector.match_replace(
                    out=sims2[:],
                    in_to_replace=m[:, :8],
                    in_values=sims[:],
                    imm_value=NEG,
                )
                nc.vector.max(out=m[:, 8:16], in_=sims2[:])
                sims3 = pool.tile([128, P], f32, tag="sims3")
                nc.vector.match_replace(
                    out=sims3[:],
                    in_to_replace=m[:, 8:16],
                    in_values=sims2[:],
                    imm_value=NEG,
                )
                nc.vector.max(out=m[:, 16:], in_=sims3[:])
                # duplicate candidates have bitwise-identical sims and sort
                # adjacent; mask every adjacent-equal rank
                dup = pool.tile([128, W - 1], f32, tag="dup")
                nc.vector.tensor_tensor(
                    out=dup[:], in0=m[:, 1:], in1=m[:, : W - 1], op=Alu.is_equal
                )
                mm = pool.tile([128, W], f32, tag="mm")
                nc.vector.tensor_copy(mm[:], m[:])
                nc.vector.scalar_tensor_tensor(
                    out=mm[:, 1:],
                    in0=dup[:],
                    scalar=NEG,
                    in1=m[:, 1:],
                    op0=Alu.mult,
                    op1=Alu.add,
                )
                mf = pool.tile([128, 8], f32, tag="mf")
                nc.vector.max(out=mf[:], in_=mm[:])
                # value extraction: for each rank k, min over candidates with
                # sims == mf[k] of (value - 2^23); exact for ints < 2^24
                candfB = pool.tile([128, P], f32, tag="candfB")
                nc.vector.tensor_scalar(
                    out=candfB[:],
                    in0=cand[:],
                    scalar1=-BIG,
                    scalar2=None,
                    op0=Alu.add,
                )
                eqc = pbig.tile([128, 8, P], f32, tag="eqc")
                nc.vector.tensor_tensor(
                    out=eqc[:, :, :],
                    in0=sims[:, None, :].to_broadcast([128, 8, P]),
                    in1=mf[:, :, None].to_broadcast([128, 8, P]),
                    op=Alu.is_equal,
                )
                selv = pbig.tile([128, 8, P], f32, tag="selv")
                nc.vector.tensor_tensor(
                    out=selv[:, :, :],
                    in0=eqc[:, :, :],
                    in1=candfB[:, None, :].to_broadcast([128, 8, P]),
                    op=Alu.mult,
                )
                vmin = pool.tile([128, 8], f32, tag="vmin")
                nc.vector.tensor_reduce(
                    out=vmin[:],
                    in_=selv[:, :, :],
                    axis=mybir.AxisListType.X,
                    op=Alu.min,
                )
                outv = pool.tile([128, K], i32, tag="outv")
                nc.vector.tensor_scalar(
                    out=outv[:],
                    in0=vmin[:],
                    scalar1=BIG,
                    scalar2=None,
                    op0=Alu.add,
                )
                nc.sync.dma_start(out=out[r, :], in_=outv[:])
    nc.compile()
    return nc


def _get_nc():
    key = (N_TOTAL, NODES_PER_CORE, N_CORES)
    if key not in _CACHE:
        _CACHE[key] = _build(*key)
    return _CACHE[key]


LAST_EXEC_TIME_NS = None


def kernel(x, neighbors, current_samples):
    import os
    from concourse.bass_utils import run_bass_kernel_spmd

    x = np.ascontiguousarray(x, dtype=np.float32)
    neighbors = np.ascontiguousarray(neighbors, dtype=np.int32)
    current_samples = np.ascontiguousarray(current_samples, dtype=np.int32)
    n = x.shape[0]
    assert n == N_TOTAL, (n, N_TOTAL)

    nc = _get_nc()
    in_maps = []
    for c in range(N_CORES):
        lo = c * NODES_PER_CORE_RAW
        hi = lo + NODES_PER_CORE_RAW
        pad = NODES_PER_CORE - NODES_PER_CORE_RAW
        nb_s = np.vstack(
            [neighbors[lo:hi], np.zeros((pad, D_NB), np.int32)]
        )
        cs_s = np.vstack(
            [current_samples[lo:hi], np.zeros((pad, K), np.int32)]
        )
        x_s = np.vstack([x[lo:hi], np.zeros((pad, D_EMB), np.float32)])
        in_maps.append(
            {
                "x": x,
                "cs_full": current_samples,
                "nb": nb_s,
                "cs_loc": cs_s,
                "x_loc": x_s,
            }
        )
    kwargs = {}
    if os.environ.get("KERNEL_TRACE") == "1":
        import tempfile

        kwargs = {"trace": True, "tmpdir": tempfile.mkdtemp(dir="/tmp")}
    res = run_bass_kernel_spmd(nc, in_maps, list(range(N_CORES)), **kwargs)
    global LAST_EXEC_TIME_NS
    LAST_EXEC_TIME_NS = res.exec_time_ns
    outs = [res.results[c]["out"][:NODES_PER_CORE_RAW] for c in range(N_CORES)]
    return np.ascontiguousarray(np.vstack(outs), dtype=np.int32)
